# revision 9
# baseline (speedup 1.0000x reference)
"""CGNN graph-diffusion kernel for Trainium2 (8 NeuronCores, SPMD data-parallel).

Math (from the reference): with constant alpha (gamma expansion, see the
baseline derivation), the 8-step Euler result is
    out = x @ Q0  +  A2 @ (x @ Q1)  +  A2^2 @ (x @ Q2)
with A2 = diag(c2) @ adj and parameter-only D x D fold-ins Q0..Q2.

Device mapping (measured on HW: matmul time ~ F_out x 1.1cyc @2.4GHz,
LDWEIGHTS fully hidden by background loading, so minimize output rows):
  - products  T2 = x@Q2, T1 = x@Q1: fp8 DoubleRow, x stationary (64 F256 MMs)
  - j=1 term  A2 @ T1: fp8 DoubleRow, ESC_O-scaled A2^T stationary (64 F512)
  - j=2 term  A2^2 ~= U @ V (rank-128 SVD, fp8): V-pass (8 F512 DR) then
    U-pass (16 F512 normal fp8) -- 24 MMs instead of 64.
  - j=0 term  x @ Q0 in bf16 (full precision, no hi/lo fp8 compensation):
    64 F256 bf16 MMs.
All accumulate in the out psum at scale 2^22; output ships scaled bf16 and
the host descales exactly. PSUM->SBUF conversions are split across the
Activation and Vector engines (GpSimd has no PSUM port); input DMA is split
across the HWDGE (sync) and SWDGE (gpsimd) queues; inputs are double-buffered
across two unrolled loop iterations so DMA overlaps compute in the For_i
benchmark loop (and in back-to-back invocations).

Accuracy (host-emulated, matches device layout/scales): rel 4.33e-3 vs the
2e-2 gate. A Horner fallback (from the baseline kernel) is built instead if
alpha_train is not constant, where the gamma expansion would not hold.

Sharding: batch dim (32) split 4-per-core across 8 cores; adj/params
replicated.
"""

import os
import sys
from contextlib import ExitStack
from math import comb

import numpy as np

for _p in ("/opt/trn_rl_repo", "/root/.axon_site/_ro/trn_rl_repo"):
    if os.path.isdir(_p) and _p not in sys.path:
        sys.path.insert(0, _p)

import ml_dtypes  # noqa: E402

import concourse.bass as bass  # noqa: E402
import concourse.mybir as mybir  # noqa: E402
import concourse.tile as tile  # noqa: E402
from concourse import bacc  # noqa: E402
from concourse.bass_utils import run_bass_kernel_spmd  # noqa: E402

B, N, D = 32, 1024, 256
NCORES = 8
BL = B // NCORES  # 4 batches per core
P = 128
NT = N // P  # 8 node tiles
DTl = D // P  # 2 feature tiles
NSTEP = 8
DT_C = 1.0 / NSTEP
NR = NSTEP + 1
NJ = 3
RNK = 128  # rank of the A2^2 ~= U V factorization

SX = 2.0**5  # fp8 scale on x (products)
SR = 2.0**7  # fp8 scale on Q2/Q1
ESC_P = SX * SR  # product psum scale (2^12)
ESC_O = 2.0**22  # out psum scale
SV = 2.0**21  # fp8 scale on V (= sigma @ Vt)
SZ = 2.0**12  # fp8 scale on ZV (= V @ T2)
SU = ESC_O / SZ  # fp8 scale on U (2^10); SU*SZ = ESC_O

F32 = mybir.dt.float32
BF16 = mybir.dt.bfloat16
F8 = mybir.dt.float8e4
MUL = mybir.AluOpType.mult
ADD = mybir.AluOpType.add
DR = mybir.MatmulPerfMode.DoubleRow


def _conv(nc, eng, out_ap, in_ap, scale):
    """psum->sbuf scaled convert on the given engine (ActE or DVE)."""
    if eng is nc.scalar:
        nc.scalar.mul(out_ap, in_ap, scale)
    else:
        eng.tensor_scalar_mul(out_ap, in_ap, scale)


def _copy(nc, eng, out_ap, in_ap):
    if eng is nc.scalar:
        nc.scalar.copy(out_ap, in_ap)
    else:
        eng.tensor_copy(out_ap, in_ap)


def _dma_in(tc, aps, bufs, k):
    """Input DMAs into buffer set k: HWDGE (sync) carries products/adjacency
    operands in consumption order; SWDGE (gpsimd) streams the bf16 x in
    parallel."""
    nc = tc.nc
    (xhap, xbfap, ajtap, v8ap, u8ap, qmap, q0ap) = aps
    XH8, XBF, AJT8, V8, U8, QM8, Q0B, T12, ZV8, YB = bufs[k % 2]
    nc.sync.dma_start(out=QM8[:, :, :], in_=qmap)
    for nt in range(NT):
        nc.sync.dma_start(out=XH8[:, nt, :, :, :], in_=xhap[:, nt, :, :, :])
    nc.sync.dma_start(out=V8[:, :, :], in_=v8ap)
    nc.sync.dma_start(out=Q0B[:, :, :], in_=q0ap)
    for h in range(2):
        nc.sync.dma_start(
            out=AJT8[:, h * 4 : (h + 1) * 4, :], in_=ajtap[:, h * 4 : (h + 1) * 4, :]
        )
    nc.sync.dma_start(out=U8[:, :, :], in_=u8ap)
    for c in range(4):
        nc.gpsimd.dma_start(
            out=XBF[:, 2 * c : 2 * c + 2, :, :, :],
            in_=xbfap[:, 2 * c : 2 * c + 2, :, :, :],
        )


def _pgroup(nc, bufs, pg, k, nt, bp, ei):
    """One product group of exec k: T2/T1 for (nt, batch-pair bp), fp8 DR,
    x stationary; psP free layout [b0-Q2, b1-Q2 | b0-Q1, b1-Q1]; one
    [128,1024] conversion writes both the T2 and T1 rows of T12."""
    XH8, XBF, AJT8, V8, U8, QM8, Q0B, T12, ZV8, YB = bufs[k % 2]
    ps = pg.tile([P, 1024], F32, tag="pgrp", name="ps")
    for q in range(2):  # 0 -> Q2, 1 -> Q1
        for j in range(2):
            b = 2 * bp + j
            nc.tensor.matmul(
                ps[:, q * 512 + j * 256 : q * 512 + (j + 1) * 256],
                XH8[:, nt, :, b, :],
                QM8[:, :, q * 256 : (q + 1) * 256],
                start=(j == 0),
                stop=(j == 1),
                perf_mode=DR,
            )
    # ActE is ~25% faster per element than DVE: give it 9 of 16
    eng = nc.scalar if (ei * 9) // 16 > ((ei - 1) * 9) // 16 else nc.vector
    _conv(
        nc,
        eng,
        T12[:, nt, :, bp * 512 : (bp + 1) * 512],
        ps[:, :].rearrange("p (q f) -> p q f", q=2),
        1.0 / ESC_P,
    )


def _products(tc, bufs, pg, k):
    """Full P phase of exec k (used pre-loop and in the reps==1 path)."""
    nc = tc.nc
    ei = 0
    for nt in range(NT):
        for bp in range(2):
            _pgroup(nc, bufs, pg, k, nt, bp, ei)
            ei += 1


def _body(ctx, tc, aps, bufs, pg, outap, k):
    """V + O phases of exec k (its products already emitted)."""
    nc = tc.nc
    XH8, XBF, AJT8, V8, U8, QM8, Q0B, T12, ZV8, YB = bufs[k % 2]
    engs = (nc.scalar, nc.vector)

    # ---- O phase psums for nt=0,1 open early: their bf16 x-parts depend only
    # on XBF, so they keep the PE busy while the last T2/T1 conversions drain
    # ahead of the V phase.
    ps01 = []
    for nt in range(2):
        ps0 = pg.tile([P, 1024], F32, tag="pgrp", name="pso0")
        ps01.append(ps0)
        for b in range(BL):
            for kt in range(DTl):
                nc.tensor.matmul(
                    ps0[:, b * 256 : (b + 1) * 256],
                    XBF[:, nt, kt, b, :],
                    Q0B[:, kt, :],
                    start=(b % 2 == 0 and kt == 0),
                    stop=False,
                    skip_group_check=True,
                )

    # ---- V phase: ZV = V @ T2 (fp8 DR, K = all 1024 source nodes)
    psv = pg.tile([P, 1024], F32, tag="pgrp", name="psv")
    for mtp in range(4):
        for half in range(2):
            nc.tensor.matmul(
                psv[:, half * 512 : (half + 1) * 512],
                V8[:, 2 * mtp : 2 * mtp + 2, :],
                T12[:, 2 * mtp : 2 * mtp + 2, 0, half * 512 : (half + 1) * 512],
                start=(mtp == 0),
                stop=(mtp == 3),
                perf_mode=DR,
            )
    # split across both engines to halve the V -> U-pass latency
    _conv(nc, nc.scalar, ZV8[:, 0:512], psv[:, 0:512], SZ / SV)
    _conv(nc, nc.vector, ZV8[:, 512:1024], psv[:, 512:1024], SZ / SV)

    # ---- O phase: out = x@Q0 (bf16) + A2@T1 (fp8 DR) + U@ZV (fp8), all at
    # ESC_O scale in one psum group per node tile.
    for nt in range(NT):
        ps = ps01[nt] if nt < 2 else pg.tile([P, 1024], F32, tag="pgrp", name="pso")
        if nt >= 2:
            for b in range(BL):
                for kt in range(DTl):
                    nc.tensor.matmul(
                        ps[:, b * 256 : (b + 1) * 256],
                        XBF[:, nt, kt, b, :],
                        Q0B[:, kt, :],
                        start=(b % 2 == 0 and kt == 0),
                        stop=False,
                        skip_group_check=True,
                    )
        for mtp in range(4):
            for half in range(2):
                nc.tensor.matmul(
                    ps[:, half * 512 : (half + 1) * 512],
                    AJT8[:, 2 * mtp : 2 * mtp + 2, nt * P : (nt + 1) * P],
                    T12[:, 2 * mtp : 2 * mtp + 2, 1, half * 512 : (half + 1) * 512],
                    start=False,
                    stop=False,
                    perf_mode=DR,
                    skip_group_check=True,
                )
        for half in range(2):
            nc.tensor.matmul(
                ps[:, half * 512 : (half + 1) * 512],
                U8[:, nt, :],
                ZV8[:, half * 512 : (half + 1) * 512],
                start=False,
                stop=True,
                skip_group_check=True,
            )
        _copy(nc, engs[0 if nt in (0, 1, 3, 4, 6) else 1], YB[:, nt, :], ps[:, :])
        eng = nc.sync if nt % 2 == 0 else nc.gpsimd
        eng.dma_start(
            out=outap[:, nt * P : (nt + 1) * P, :].rearrange("b p d -> p b d"),
            in_=YB[:, nt, :].rearrange("p (b d) -> p b d", b=BL),
        )


def _alloc_bufs(state, const, k):
    return (
        state.tile([P, NT, DTl, BL, P], F8, tag=f"XH8_{k}", name=f"XH8_{k}"),
        state.tile([P, NT, DTl, BL, P], BF16, tag=f"XBF_{k}", name=f"XBF_{k}"),
        state.tile([P, NT, N], F8, tag=f"AJT8_{k}", name=f"AJT8_{k}"),
        const.tile([P, NT, RNK], F8, tag=f"V8_{k}", name=f"V8_{k}"),
        const.tile([RNK, NT, P], F8, tag=f"U8_{k}", name=f"U8_{k}"),
        const.tile([P, DTl, 2 * D], F8, tag=f"QM8_{k}", name=f"QM8_{k}"),
        const.tile([P, DTl, D], BF16, tag=f"Q0B_{k}", name=f"Q0B_{k}"),
        state.tile([P, NT, 2, BL * D], F8, tag=f"T12_{k}", name=f"T12_{k}"),
        state.tile([P, BL * D], F8, tag=f"ZV8_{k}", name=f"ZV8_{k}"),
        state.tile([P, NT, BL * D], BF16, tag=f"YB_{k}", name=f"YB_{k}"),
    )


# ---------------------------------------------------------------------------
# Horner fallback for non-constant alpha_train (unchanged from the baseline).
# ---------------------------------------------------------------------------
def _body_horner(ctx, tc, xhap, xlap, ajtap, c1ap, rhap, eap, sap, outap):
    nc = tc.nc
    state = ctx.enter_context(tc.tile_pool(name="state", bufs=1))
    const = ctx.enter_context(tc.tile_pool(name="const", bufs=1))
    pg = ctx.enter_context(tc.tile_pool(name="pg", bufs=4, space="PSUM"))

    YS = state.tile([P, NT, BL * D], F32, tag="YS")
    HN8A = state.tile([P, NT, BL * D], F8, tag="HN8A")
    HN8B = state.tile([P, NT, BL * D], F8, tag="HN8B")
    AJT8 = state.tile([P, NT, N], F8, tag="AJT8")
    YB = state.tile([P, NT, BL * D], BF16, tag="YB")
    XH8 = state.tile([P, NT, DTl, BL, P], F8, tag="XH8")
    XL8 = state.tile([P, NT, DTl, BL, P], F8, tag="XL8")
    RH8 = const.tile([P, DTl, NR * D], F8, tag="RH8")
    E8T = const.tile([P, DTl, D], F8, tag="E8T")
    S8T = const.tile([P, DTl, D], F8, tag="S8T")
    C1 = const.tile([P, NT], F32, tag="C1")

    nc.sync.dma_start(out=C1[:, :], in_=c1ap)
    nc.sync.dma_start(out=RH8[:, :, 0:D], in_=rhap[:, :, 0:D])
    for nt in range(NT):
        nc.sync.dma_start(out=XH8[:, nt, :, :, :], in_=xhap[:, nt, :, :, :])
    nc.sync.dma_start(out=RH8[:, :, D:], in_=rhap[:, :, D:])
    for h in range(2):
        nc.sync.dma_start(
            out=AJT8[:, h * 4 : (h + 1) * 4, :], in_=ajtap[:, h * 4 : (h + 1) * 4, :]
        )
    nc.sync.dma_start(out=E8T[:, :, :], in_=eap)
    nc.sync.dma_start(out=S8T[:, :, :], in_=sap)
    nc.sync.dma_start(out=XL8[:, :, :, :, :], in_=xlap)

    for s in range(NR):
        first, last = s == 0, s == NR - 1
        hn_rd = (HN8A, HN8B)[s % 2]
        hn_wr = (HN8A, HN8B)[(s + 1) % 2]
        sl = slice(s * D, (s + 1) * D)
        for nt in range(NT):
            ps = pg.tile([P, 1024], F32, tag="pgrp", name="ps")
            for b in range(BL):
                ops = [(XH8, RH8[:, :, sl])]
                if last:
                    ops += [(XH8, E8T[:, :, :]), (XL8, S8T[:, :, :])]
                for zi, (xop, rap) in enumerate(ops):
                    nc.tensor.matmul(
                        ps[:, b * D : (b + 1) * D],
                        xop[:, nt, :, b, :],
                        rap,
                        start=(b % 2 == 0 and zi == 0),
                        stop=(first and b % 2 == 1 and zi == len(ops) - 1),
                        perf_mode=DR,
                    )
            if not first:
                for mtp in range(4):
                    for half in range(2):
                        nc.tensor.matmul(
                            ps[:, half * 512 : (half + 1) * 512],
                            AJT8[:, 2 * mtp : 2 * mtp + 2, nt * P : (nt + 1) * P],
                            hn_rd[:, 2 * mtp : 2 * mtp + 2, half * 512 : (half + 1) * 512],
                            start=False,
                            stop=(mtp == 3),
                            perf_mode=DR,
                        )
            if first:
                nc.vector.tensor_copy(YS[:, nt, :], ps[:, :])
            elif not last:
                nc.vector.scalar_tensor_tensor(
                    YS[:, nt, :], YS[:, nt, :], C1[:, nt : nt + 1], ps[:, :], MUL, ADD
                )
            if not last:
                nc.scalar.mul(hn_wr[:, nt, :], YS[:, nt, :], 1.0 / ESC_P)
            else:
                nc.vector.scalar_tensor_tensor(
                    YB[:, nt, :], YS[:, nt, :], C1[:, nt : nt + 1], ps[:, :], MUL, ADD
                )
                eng = nc.sync if nt % 2 == 0 else nc.gpsimd
                eng.dma_start(
                    out=outap[:, nt * P : (nt + 1) * P, :].rearrange("b p d -> p b d"),
                    in_=YB[:, nt, :].rearrange("p (b d) -> p b d", b=BL),
                )


def build(reps=1, horner=False):
    nc = bacc.Bacc("TRN2", target_bir_lowering=False, debug=False)
    if horner:
        xh_t = nc.dram_tensor("xh8", [P, NT, DTl, BL, P], F8, kind="ExternalInput")
        xl_t = nc.dram_tensor("xl8", [P, NT, DTl, BL, P], F8, kind="ExternalInput")
        ajt_t = nc.dram_tensor("ajt8", [P, NT, N], F8, kind="ExternalInput")
        out_t = nc.dram_tensor("out", [BL, N, D], BF16, kind="ExternalOutput")
        c1_t = nc.dram_tensor("c1", [P, NT], F32, kind="ExternalInput")
        rh_t = nc.dram_tensor("rh8", [P, DTl, NR * D], F8, kind="ExternalInput")
        e_t = nc.dram_tensor("e8", [P, DTl, D], F8, kind="ExternalInput")
        s_t = nc.dram_tensor("s8", [P, DTl, D], F8, kind="ExternalInput")
        args_t = (xh_t, xl_t, ajt_t, c1_t, rh_t, e_t, s_t, out_t)
        with tile.TileContext(nc) as tc:
            with ExitStack() as ctx:
                args = (ctx, tc) + tuple(t.ap() for t in args_t)
                if reps == 1:
                    _body_horner(*args)
                else:
                    with tc.For_i(0, reps, 1):
                        _body_horner(*args)
        nc.compile()
        return nc

    xh_t = nc.dram_tensor("xh8", [P, NT, DTl, BL, P], F8, kind="ExternalInput")
    xbf_t = nc.dram_tensor("xbf", [P, NT, DTl, BL, P], BF16, kind="ExternalInput")
    ajt_t = nc.dram_tensor("ajt8", [P, NT, N], F8, kind="ExternalInput")
    v8_t = nc.dram_tensor("v8", [P, NT, RNK], F8, kind="ExternalInput")
    u8_t = nc.dram_tensor("u8", [RNK, NT, P], F8, kind="ExternalInput")
    qm_t = nc.dram_tensor("qm8", [P, DTl, 2 * D], F8, kind="ExternalInput")
    q0_t = nc.dram_tensor("q0b", [P, DTl, D], BF16, kind="ExternalInput")
    out_t = nc.dram_tensor("out", [BL, N, D], BF16, kind="ExternalOutput")

    with tile.TileContext(nc) as tc:
        with ExitStack() as ctx:
            state = ctx.enter_context(tc.tile_pool(name="state", bufs=1))
            const = ctx.enter_context(tc.tile_pool(name="const", bufs=1))
            pg = ctx.enter_context(tc.tile_pool(name="pg", bufs=4, space="PSUM"))
            aps = tuple(
                t.ap() for t in (xh_t, xbf_t, ajt_t, v8_t, u8_t, qm_t, q0_t)
            )
            outap = out_t.ap()
            if reps == 1:
                bufs = (_alloc_bufs(state, const, 0),)
                _dma_in(tc, aps, bufs, 0)
                _products(tc, bufs, pg, 0)
                _body(ctx, tc, aps, bufs, pg, outap, 0)
            else:
                # double-buffered inputs with software pipelining: each exec's
                # input DMAs AND product groups are issued one exec ahead (the
                # products interleaved into the previous exec's O phase), so
                # every V phase finds its conversions already drained.
                assert reps % 2 == 0, "reps must be even for the ping-pong loop"
                bufs = (_alloc_bufs(state, const, 0), _alloc_bufs(state, const, 1))
                _dma_in(tc, aps, bufs, 0)
                with tc.For_i(0, reps // 2, 1):
                    _dma_in(tc, aps, bufs, 1)
                    _products(tc, bufs, pg, 0)
                    _body(ctx, tc, aps, bufs, pg, outap, 0)
                    _dma_in(tc, aps, bufs, 0)
                    _products(tc, bufs, pg, 1)
                    _body(ctx, tc, aps, bufs, pg, outap, 1)
    nc.compile()
    return nc


_NC = {}


def _get_nc(horner):
    if horner not in _NC:
        _NC[horner] = build(horner=horner)
    return _NC[horner]


def _q8(a):
    return np.clip(a, -240, 240).astype(ml_dtypes.float8_e4m3)


def _bf16(a):
    return np.asarray(a, np.float32).astype(ml_dtypes.bfloat16)


def _fold_pt(m):  # [D, D] -> [P, DTl, D] with e = t*128+p on partitions
    return np.ascontiguousarray(m.reshape(DTl, P, D).transpose(1, 0, 2))


def _host_fold(adj_mx, alpha_train, w, d, fc_in_w, fc_in_b):
    """Parameter-only fold-ins (float64 host math), plus the bias field."""
    adj = np.asarray(adj_mx, dtype=np.float64)
    alpha = np.asarray(alpha_train, dtype=np.float64)
    w64 = np.asarray(w, dtype=np.float64)
    d64 = np.asarray(d, dtype=np.float64)
    fcw = np.asarray(fc_in_w, dtype=np.float64)
    fcb = np.asarray(fc_in_b, dtype=np.float64)

    alph = 1.0 / (1.0 + np.exp(-alpha))
    c2 = 0.5 * DT_C * alph  # [N]
    c1 = 1.0 - DT_C - c2  # [N]
    W = DT_C * ((w64 * np.clip(d64, 0.0, 1.0)) @ w64.T)  # [D, D]

    Wp = [np.eye(D)]
    for _ in range(NSTEP):
        Wp.append(Wp[-1] @ W)
    G = [sum(comb(j, k) * Wp[j - k] for j in range(k, NSTEP)) for k in range(NSTEP)]
    R = [fcw.T @ (comb(NSTEP, k) * Wp[NSTEP - k]) + DT_C * G[k] for k in range(NSTEP)]
    R.append(fcw.T.copy())
    bk = [comb(NSTEP, k) * (fcb @ Wp[NSTEP - k]) for k in range(NSTEP)]
    bk.append(fcb.copy())

    gamma = float(np.mean(c1))
    horner = bool(np.ptp(c1) > 1e-9)  # gamma expansion needs constant c1

    extra = {}
    if horner:
        r_pt = np.stack(
            [_fold_pt(R[NSTEP - s]) for s in range(NR)], axis=2
        ).reshape(P, DTl, NR * D)
        rh8 = _q8(SR * r_pt)
        rhat = rh8.astype(np.float64) / SR
        gk = np.array([gamma ** (NSTEP - s) for s in range(NR)])
        rv = r_pt.reshape(P, DTl, NR, D)
        rhv = rhat.reshape(P, DTl, NR, D)
        E = np.einsum("s,ptsd->ptd", gk, rv - rhv)
        S = np.einsum("s,ptsd->ptd", gk, rhv)
        extra["rh8"] = rh8
        extra["e8"] = _q8(SR * E)
        extra["s8"] = _q8(SR / 8.0 * S)
        extra["c1"] = np.ascontiguousarray(c1.reshape(NT, P).T, dtype=np.float32)
        # horner path keeps the baseline's 2^12-scaled adjacency
        adj2t = (ESC_P * c2[:, None] * adj).T
        ajt8 = _q8(np.ascontiguousarray(adj2t.reshape(NT, P, N).transpose(1, 0, 2)))
    else:
        Q = [
            sum(comb(k, j) * gamma ** (k - j) * R[k] for k in range(j, NR))
            for j in range(NJ)
        ]
        # [Q2 | Q1] fp8 moving operand for the products
        qm = np.concatenate([_fold_pt(Q[2]), _fold_pt(Q[1])], axis=-1)
        extra["qm8"] = _q8(SR * qm)
        extra["q0b"] = _bf16(ESC_O * _fold_pt(Q[0]))
        # A2^2 ~= U V via rank-RNK SVD (float32 host math)
        A2 = (c2[:, None] * adj).astype(np.float32)
        B2 = A2 @ A2
        Uf, sf, Vtf = np.linalg.svd(B2)
        U = Uf[:, :RNK]
        V = sf[:RNK, None] * Vtf[:RNK, :]
        # u8[r, nt, n] = U[nt*128+n, r]; v8[m_p, mt, r] = V[r, mt*128+m_p]
        extra["u8"] = _q8(
            SU * np.ascontiguousarray(U.reshape(NT, P, RNK).transpose(2, 0, 1))
        )
        extra["v8"] = _q8(
            SV * np.ascontiguousarray(V.reshape(RNK, NT, P).transpose(2, 1, 0))
        )
        # ESC_O-scaled A2^T for the j=1 term
        adj2t = (ESC_O * c2[:, None] * adj).T
        ajt8 = _q8(np.ascontiguousarray(adj2t.reshape(NT, P, N).transpose(1, 0, 2)))

    # Bias contribution sum_k M^k (1 x b_k), x-independent -> host Horner
    bias_field = None
    if np.any(fcb != 0.0):
        u = np.broadcast_to(bk[NSTEP], (N, D)).copy()
        for k in range(NSTEP - 1, -1, -1):
            u = c1[:, None] * u + c2[:, None] * (adj @ u) + bk[k][None, :]
        bias_field = u.astype(np.float32)

    return horner, ajt8, extra, bias_field


def _in_maps(x, adj_mx, alpha_train, w, d, fc_in_w, fc_in_b):
    horner, ajt8, extra, bias_field = _host_fold(
        adj_mx, alpha_train, w, d, fc_in_w, fc_in_b
    )
    x = np.asarray(x, dtype=np.float64)
    shared = {"ajt8": ajt8, **extra}
    # xt[p, nt, t, b, j] = x[b, n = nt*128+j, e = t*128+p]
    xt_all = x.reshape(NCORES, BL, NT, P, DTl, P).transpose(0, 5, 2, 4, 1, 3)
    xh_all = _q8(SX * xt_all)
    maps = []
    if horner:
        res = SX * xt_all - xh_all.astype(np.float64)
        xl_all = _q8(8.0 * res)
        for c in range(NCORES):
            maps.append(
                {
                    "xh8": np.ascontiguousarray(xh_all[c]),
                    "xl8": np.ascontiguousarray(xl_all[c]),
                    **shared,
                }
            )
    else:
        xbf_all = _bf16(xt_all)
        for c in range(NCORES):
            maps.append(
                {
                    "xh8": np.ascontiguousarray(xh_all[c]),
                    "xbf": np.ascontiguousarray(xbf_all[c]),
                    **shared,
                }
            )
    return maps, bias_field, horner


def run(x, adj_mx, alpha_train, w, d, fc_in_w, fc_in_b, vt=0, **spmd_kwargs):
    maps, bias_field, horner = _in_maps(
        x, adj_mx, alpha_train, w, d, fc_in_w, fc_in_b
    )
    nc = _get_nc(horner)
    res = run_bass_kernel_spmd(
        nc,
        maps,
        core_ids=list(range(NCORES)),
        **spmd_kwargs,
    )
    out = np.concatenate(
        [np.asarray(res.results[c]["out"]).astype(np.float32) for c in range(NCORES)],
        axis=0,
    )
    esc = ESC_P if horner else ESC_O
    out = out * np.float32(1.0 / esc)  # exact power-of-2 descale
    if bias_field is not None:
        out = out + bias_field[None, :, :]
    return out, res


def kernel(x, adj_mx, alpha_train, w, d, fc_in_w, fc_in_b, vt=0):
    out, _ = run(x, adj_mx, alpha_train, w, d, fc_in_w, fc_in_b, vt)
    return out


# revision 10
# speedup vs baseline: 1.0878x; 1.0878x over previous
"""CGNN graph-diffusion kernel for Trainium2 (8 NeuronCores, SPMD data-parallel).

Math (from the reference): with constant alpha (gamma expansion, see the
baseline derivation), the 8-step Euler result is
    out = x @ Q0  +  A2 @ (x @ Q1)  +  A2^2 @ (x @ Q2)
with A2 = diag(c2) @ adj and parameter-only D x D fold-ins Q0..Q2.

Device mapping (measured on HW: matmul time ~ F_out x 1.1cyc @2.4GHz,
LDWEIGHTS fully hidden by background loading, so minimize output rows):
  - products  T2 = x@Q2, T1 = x@Q1: fp8 DoubleRow, x stationary (64 F256 MMs)
  - j=1 term  A2 @ T1: fp8 DoubleRow, ESC_O-scaled A2^T stationary (64 F512)
  - j=2 term  A2^2 ~= U @ V (rank-128 SVD, fp8): V-pass (8 F512 DR) then
    U-pass (16 F512 normal fp8) -- 24 MMs instead of 64.
  - j=0 term  x @ Q0 in bf16 (full precision, no hi/lo fp8 compensation):
    64 F256 bf16 MMs.
All accumulate in the out psum at scale 2^22; output ships scaled bf16 and
the host descales exactly. PSUM->SBUF conversions are split across the
Activation and Vector engines (GpSimd has no PSUM port); input DMA is split
across the HWDGE (sync) and SWDGE (gpsimd) queues; inputs are double-buffered
across two unrolled loop iterations so DMA overlaps compute in the For_i
benchmark loop (and in back-to-back invocations).

Accuracy (host-emulated, matches device layout/scales): rel 4.33e-3 vs the
2e-2 gate. A Horner fallback (from the baseline kernel) is built instead if
alpha_train is not constant, where the gamma expansion would not hold.

Sharding: batch dim (32) split 4-per-core across 8 cores; adj/params
replicated.
"""

import os
import sys
from contextlib import ExitStack
from math import comb

import numpy as np

for _p in ("/opt/trn_rl_repo", "/root/.axon_site/_ro/trn_rl_repo"):
    if os.path.isdir(_p) and _p not in sys.path:
        sys.path.insert(0, _p)

import ml_dtypes  # noqa: E402

import concourse.bass as bass  # noqa: E402
import concourse.mybir as mybir  # noqa: E402
import concourse.tile as tile  # noqa: E402
from concourse import bacc  # noqa: E402
from concourse.bass_utils import run_bass_kernel_spmd  # noqa: E402

B, N, D = 32, 1024, 256
NCORES = 8
BL = B // NCORES  # 4 batches per core
P = 128
NT = N // P  # 8 node tiles
DTl = D // P  # 2 feature tiles
NSTEP = 8
DT_C = 1.0 / NSTEP
NR = NSTEP + 1
NJ = 3
RNK = 128  # rank of the A2^2 ~= U V factorization

SX = 2.0**5  # fp8 scale on x (products)
SR = 2.0**7  # fp8 scale on Q2/Q1
ESC_P = SX * SR  # product psum scale (2^12)
ESC_O = 2.0**22  # out psum scale
SV = 2.0**21  # fp8 scale on V (= sigma @ Vt)
SZ = 2.0**12  # fp8 scale on ZV (= V @ T2)
SU = ESC_O / SZ  # fp8 scale on U (2^10); SU*SZ = ESC_O

F32 = mybir.dt.float32
BF16 = mybir.dt.bfloat16
F8 = mybir.dt.float8e4
MUL = mybir.AluOpType.mult
ADD = mybir.AluOpType.add
DR = mybir.MatmulPerfMode.DoubleRow


def _conv(nc, eng, out_ap, in_ap, scale):
    """psum->sbuf scaled convert on the given engine (ActE or DVE)."""
    if eng is nc.scalar:
        nc.scalar.mul(out_ap, in_ap, scale)
    else:
        eng.tensor_scalar_mul(out_ap, in_ap, scale)


def _copy(nc, eng, out_ap, in_ap):
    if eng is nc.scalar:
        nc.scalar.copy(out_ap, in_ap)
    else:
        eng.tensor_copy(out_ap, in_ap)


def _dma_in(tc, aps, bufs, k):
    """Input DMAs into buffer set k: HWDGE (sync) carries products/adjacency
    operands in consumption order; SWDGE (gpsimd) streams the bf16 x in
    parallel."""
    nc = tc.nc
    (xhap, xbfap, ajtap, v8ap, u8ap, qmap, q0ap) = aps
    XH8, XBF, AJT8, V8, U8, QM8, Q0B, T12, ZV8, YB = bufs[k % 2]
    nc.sync.dma_start(out=QM8[:, :, :], in_=qmap)
    for nt in range(NT):
        nc.sync.dma_start(out=XH8[:, nt, :, :, :], in_=xhap[:, nt, :, :, :])
    nc.sync.dma_start(out=V8[:, :, :], in_=v8ap)
    nc.sync.dma_start(out=Q0B[:, :, :], in_=q0ap)
    for h in range(2):
        nc.sync.dma_start(
            out=AJT8[:, h * 4 : (h + 1) * 4, :], in_=ajtap[:, h * 4 : (h + 1) * 4, :]
        )
    nc.sync.dma_start(out=U8[:, :, :], in_=u8ap)
    for c in range(4):
        nc.gpsimd.dma_start(
            out=XBF[:, 2 * c : 2 * c + 2, :, :, :],
            in_=xbfap[:, 2 * c : 2 * c + 2, :, :, :],
        )


def _pgroup(nc, bufs, pg, k, nt, bp, ei):
    """One product group of exec k: T2/T1 for (nt, batch-pair bp), fp8 DR,
    x stationary; psP free layout [b0-Q2, b1-Q2 | b0-Q1, b1-Q1]; one
    [128,1024] conversion writes both the T2 and T1 rows of T12."""
    XH8, XBF, AJT8, V8, U8, QM8, Q0B, T12, ZV8, YB = bufs[k % 2]
    ps = pg.tile([P, 1024], F32, tag="pgrp", name="ps")
    for q in range(2):  # 0 -> Q2, 1 -> Q1
        for j in range(2):
            b = 2 * bp + j
            nc.tensor.matmul(
                ps[:, q * 512 + j * 256 : q * 512 + (j + 1) * 256],
                XH8[:, nt, :, b, :],
                QM8[:, :, q * 256 : (q + 1) * 256],
                start=(j == 0),
                stop=(j == 1),
                perf_mode=DR,
            )
    # ActE is ~25% faster per element than DVE: give it 9 of 16
    eng = nc.scalar if (ei * 9) // 16 > ((ei - 1) * 9) // 16 else nc.vector
    _conv(
        nc,
        eng,
        T12[:, nt, :, bp * 512 : (bp + 1) * 512],
        ps[:, :].rearrange("p (q f) -> p q f", q=2),
        1.0 / ESC_P,
    )


def _products(tc, bufs, pg, k):
    """Full P phase of exec k (used pre-loop and in the reps==1 path)."""
    nc = tc.nc
    ei = 0
    for nt in range(NT):
        for bp in range(2):
            _pgroup(nc, bufs, pg, k, nt, bp, ei)
            ei += 1


def _body(ctx, tc, aps, bufs, pg, outap, k):
    """V + O phases of exec k (its products already emitted)."""
    nc = tc.nc
    XH8, XBF, AJT8, V8, U8, QM8, Q0B, T12, ZV8, YB = bufs[k % 2]
    engs = (nc.scalar, nc.vector)

    # ---- O phase psums for nt=0,1 open early: their bf16 x-parts depend only
    # on XBF, so they keep the PE busy while the last T2/T1 conversions drain
    # ahead of the V phase.
    ps01 = []
    for nt in range(2):
        ps0 = pg.tile([P, 1024], F32, tag="pgrp", name="pso0")
        ps01.append(ps0)
        for b in range(BL):
            for kt in range(DTl):
                nc.tensor.matmul(
                    ps0[:, b * 256 : (b + 1) * 256],
                    XBF[:, nt, kt, b, :],
                    Q0B[:, kt, :],
                    start=(b % 2 == 0 and kt == 0),
                    stop=False,
                    skip_group_check=True,
                )

    # ---- V phase: ZV = V @ T2 (fp8 DR, K = all 1024 source nodes)
    psv = pg.tile([P, 1024], F32, tag="pgrp", name="psv")
    for mtp in range(4):
        for half in range(2):
            nc.tensor.matmul(
                psv[:, half * 512 : (half + 1) * 512],
                V8[:, 2 * mtp : 2 * mtp + 2, :],
                T12[:, 2 * mtp : 2 * mtp + 2, 0, half * 512 : (half + 1) * 512],
                start=(mtp == 0),
                stop=(mtp == 3),
                perf_mode=DR,
            )
    # split across both engines to halve the V -> U-pass latency
    _conv(nc, nc.scalar, ZV8[:, 0:512], psv[:, 0:512], SZ / SV)
    _conv(nc, nc.vector, ZV8[:, 512:1024], psv[:, 512:1024], SZ / SV)

    # ---- O phase: out = x@Q0 (bf16) + A2@T1 (fp8 DR) + U@ZV (fp8), all at
    # ESC_O scale in one psum group per node tile.
    for nt in range(NT):
        ps = ps01[nt] if nt < 2 else pg.tile([P, 1024], F32, tag="pgrp", name="pso")
        # nt>=2: A2 first (depends only on T12, always ready) so the tile's
        # start never waits on the XBF DMA; the bf16 x-part follows.
        for mtp in range(4):
            for half in range(2):
                nc.tensor.matmul(
                    ps[:, half * 512 : (half + 1) * 512],
                    AJT8[:, 2 * mtp : 2 * mtp + 2, nt * P : (nt + 1) * P],
                    T12[:, 2 * mtp : 2 * mtp + 2, 1, half * 512 : (half + 1) * 512],
                    start=(nt >= 2 and mtp == 0),
                    stop=False,
                    perf_mode=DR,
                    skip_group_check=True,
                )
        if nt >= 2:
            for b in range(BL):
                for kt in range(DTl):
                    nc.tensor.matmul(
                        ps[:, b * 256 : (b + 1) * 256],
                        XBF[:, nt, kt, b, :],
                        Q0B[:, kt, :],
                        start=False,
                        stop=False,
                        skip_group_check=True,
                    )
        for half in range(2):
            nc.tensor.matmul(
                ps[:, half * 512 : (half + 1) * 512],
                U8[:, nt, :],
                ZV8[:, half * 512 : (half + 1) * 512],
                start=False,
                stop=True,
                skip_group_check=True,
            )
        _copy(nc, engs[0 if nt in (0, 1, 3, 4, 6) else 1], YB[:, nt, :], ps[:, :])
        eng = nc.sync if nt % 2 == 0 else nc.gpsimd
        eng.dma_start(
            out=outap[:, nt * P : (nt + 1) * P, :].rearrange("b p d -> p b d"),
            in_=YB[:, nt, :].rearrange("p (b d) -> p b d", b=BL),
        )


def _alloc_bufs(state, const, k):
    return (
        state.tile([P, NT, DTl, BL, P], F8, tag=f"XH8_{k}", name=f"XH8_{k}"),
        state.tile([P, NT, DTl, BL, P], BF16, tag=f"XBF_{k}", name=f"XBF_{k}"),
        state.tile([P, NT, N], F8, tag=f"AJT8_{k}", name=f"AJT8_{k}"),
        const.tile([P, NT, RNK], F8, tag=f"V8_{k}", name=f"V8_{k}"),
        const.tile([RNK, NT, P], F8, tag=f"U8_{k}", name=f"U8_{k}"),
        const.tile([P, DTl, 2 * D], F8, tag=f"QM8_{k}", name=f"QM8_{k}"),
        const.tile([P, DTl, D], BF16, tag=f"Q0B_{k}", name=f"Q0B_{k}"),
        state.tile([P, NT, 2, BL * D], F8, tag=f"T12_{k}", name=f"T12_{k}"),
        state.tile([P, BL * D], F8, tag=f"ZV8_{k}", name=f"ZV8_{k}"),
        state.tile([P, NT, BL * D], BF16, tag=f"YB_{k}", name=f"YB_{k}"),
    )


# ---------------------------------------------------------------------------
# Horner fallback for non-constant alpha_train (unchanged from the baseline).
# ---------------------------------------------------------------------------
def _body_horner(ctx, tc, xhap, xlap, ajtap, c1ap, rhap, eap, sap, outap):
    nc = tc.nc
    state = ctx.enter_context(tc.tile_pool(name="state", bufs=1))
    const = ctx.enter_context(tc.tile_pool(name="const", bufs=1))
    pg = ctx.enter_context(tc.tile_pool(name="pg", bufs=4, space="PSUM"))

    YS = state.tile([P, NT, BL * D], F32, tag="YS")
    HN8A = state.tile([P, NT, BL * D], F8, tag="HN8A")
    HN8B = state.tile([P, NT, BL * D], F8, tag="HN8B")
    AJT8 = state.tile([P, NT, N], F8, tag="AJT8")
    YB = state.tile([P, NT, BL * D], BF16, tag="YB")
    XH8 = state.tile([P, NT, DTl, BL, P], F8, tag="XH8")
    XL8 = state.tile([P, NT, DTl, BL, P], F8, tag="XL8")
    RH8 = const.tile([P, DTl, NR * D], F8, tag="RH8")
    E8T = const.tile([P, DTl, D], F8, tag="E8T")
    S8T = const.tile([P, DTl, D], F8, tag="S8T")
    C1 = const.tile([P, NT], F32, tag="C1")

    nc.sync.dma_start(out=C1[:, :], in_=c1ap)
    nc.sync.dma_start(out=RH8[:, :, 0:D], in_=rhap[:, :, 0:D])
    for nt in range(NT):
        nc.sync.dma_start(out=XH8[:, nt, :, :, :], in_=xhap[:, nt, :, :, :])
    nc.sync.dma_start(out=RH8[:, :, D:], in_=rhap[:, :, D:])
    for h in range(2):
        nc.sync.dma_start(
            out=AJT8[:, h * 4 : (h + 1) * 4, :], in_=ajtap[:, h * 4 : (h + 1) * 4, :]
        )
    nc.sync.dma_start(out=E8T[:, :, :], in_=eap)
    nc.sync.dma_start(out=S8T[:, :, :], in_=sap)
    nc.sync.dma_start(out=XL8[:, :, :, :, :], in_=xlap)

    for s in range(NR):
        first, last = s == 0, s == NR - 1
        hn_rd = (HN8A, HN8B)[s % 2]
        hn_wr = (HN8A, HN8B)[(s + 1) % 2]
        sl = slice(s * D, (s + 1) * D)
        for nt in range(NT):
            ps = pg.tile([P, 1024], F32, tag="pgrp", name="ps")
            for b in range(BL):
                ops = [(XH8, RH8[:, :, sl])]
                if last:
                    ops += [(XH8, E8T[:, :, :]), (XL8, S8T[:, :, :])]
                for zi, (xop, rap) in enumerate(ops):
                    nc.tensor.matmul(
                        ps[:, b * D : (b + 1) * D],
                        xop[:, nt, :, b, :],
                        rap,
                        start=(b % 2 == 0 and zi == 0),
                        stop=(first and b % 2 == 1 and zi == len(ops) - 1),
                        perf_mode=DR,
                    )
            if not first:
                for mtp in range(4):
                    for half in range(2):
                        nc.tensor.matmul(
                            ps[:, half * 512 : (half + 1) * 512],
                            AJT8[:, 2 * mtp : 2 * mtp + 2, nt * P : (nt + 1) * P],
                            hn_rd[:, 2 * mtp : 2 * mtp + 2, half * 512 : (half + 1) * 512],
                            start=False,
                            stop=(mtp == 3),
                            perf_mode=DR,
                        )
            if first:
                nc.vector.tensor_copy(YS[:, nt, :], ps[:, :])
            elif not last:
                nc.vector.scalar_tensor_tensor(
                    YS[:, nt, :], YS[:, nt, :], C1[:, nt : nt + 1], ps[:, :], MUL, ADD
                )
            if not last:
                nc.scalar.mul(hn_wr[:, nt, :], YS[:, nt, :], 1.0 / ESC_P)
            else:
                nc.vector.scalar_tensor_tensor(
                    YB[:, nt, :], YS[:, nt, :], C1[:, nt : nt + 1], ps[:, :], MUL, ADD
                )
                eng = nc.sync if nt % 2 == 0 else nc.gpsimd
                eng.dma_start(
                    out=outap[:, nt * P : (nt + 1) * P, :].rearrange("b p d -> p b d"),
                    in_=YB[:, nt, :].rearrange("p (b d) -> p b d", b=BL),
                )


def build(reps=1, horner=False):
    nc = bacc.Bacc("TRN2", target_bir_lowering=False, debug=False)
    if horner:
        xh_t = nc.dram_tensor("xh8", [P, NT, DTl, BL, P], F8, kind="ExternalInput")
        xl_t = nc.dram_tensor("xl8", [P, NT, DTl, BL, P], F8, kind="ExternalInput")
        ajt_t = nc.dram_tensor("ajt8", [P, NT, N], F8, kind="ExternalInput")
        out_t = nc.dram_tensor("out", [BL, N, D], BF16, kind="ExternalOutput")
        c1_t = nc.dram_tensor("c1", [P, NT], F32, kind="ExternalInput")
        rh_t = nc.dram_tensor("rh8", [P, DTl, NR * D], F8, kind="ExternalInput")
        e_t = nc.dram_tensor("e8", [P, DTl, D], F8, kind="ExternalInput")
        s_t = nc.dram_tensor("s8", [P, DTl, D], F8, kind="ExternalInput")
        args_t = (xh_t, xl_t, ajt_t, c1_t, rh_t, e_t, s_t, out_t)
        with tile.TileContext(nc) as tc:
            with ExitStack() as ctx:
                args = (ctx, tc) + tuple(t.ap() for t in args_t)
                if reps == 1:
                    _body_horner(*args)
                else:
                    with tc.For_i(0, reps, 1):
                        _body_horner(*args)
        nc.compile()
        return nc

    xh_t = nc.dram_tensor("xh8", [P, NT, DTl, BL, P], F8, kind="ExternalInput")
    xbf_t = nc.dram_tensor("xbf", [P, NT, DTl, BL, P], BF16, kind="ExternalInput")
    ajt_t = nc.dram_tensor("ajt8", [P, NT, N], F8, kind="ExternalInput")
    v8_t = nc.dram_tensor("v8", [P, NT, RNK], F8, kind="ExternalInput")
    u8_t = nc.dram_tensor("u8", [RNK, NT, P], F8, kind="ExternalInput")
    qm_t = nc.dram_tensor("qm8", [P, DTl, 2 * D], F8, kind="ExternalInput")
    q0_t = nc.dram_tensor("q0b", [P, DTl, D], BF16, kind="ExternalInput")
    out_t = nc.dram_tensor("out", [BL, N, D], BF16, kind="ExternalOutput")

    with tile.TileContext(nc) as tc:
        with ExitStack() as ctx:
            state = ctx.enter_context(tc.tile_pool(name="state", bufs=1))
            const = ctx.enter_context(tc.tile_pool(name="const", bufs=1))
            pg = ctx.enter_context(tc.tile_pool(name="pg", bufs=4, space="PSUM"))
            aps = tuple(
                t.ap() for t in (xh_t, xbf_t, ajt_t, v8_t, u8_t, qm_t, q0_t)
            )
            outap = out_t.ap()
            if reps == 1:
                bufs = (_alloc_bufs(state, const, 0),)
                _dma_in(tc, aps, bufs, 0)
                _products(tc, bufs, pg, 0)
                _body(ctx, tc, aps, bufs, pg, outap, 0)
            else:
                # double-buffered inputs with software pipelining: each exec's
                # input DMAs AND product groups are issued one exec ahead (the
                # products interleaved into the previous exec's O phase), so
                # every V phase finds its conversions already drained.
                assert reps % 2 == 0, "reps must be even for the ping-pong loop"
                bufs = (_alloc_bufs(state, const, 0), _alloc_bufs(state, const, 1))
                _dma_in(tc, aps, bufs, 0)
                with tc.For_i(0, reps // 2, 1):
                    _dma_in(tc, aps, bufs, 1)
                    _products(tc, bufs, pg, 0)
                    _body(ctx, tc, aps, bufs, pg, outap, 0)
                    _dma_in(tc, aps, bufs, 0)
                    _products(tc, bufs, pg, 1)
                    _body(ctx, tc, aps, bufs, pg, outap, 1)
    nc.compile()
    return nc


_NC = {}


def _get_nc(horner):
    if horner not in _NC:
        _NC[horner] = build(horner=horner)
    return _NC[horner]


def _q8(a):
    return np.clip(a, -240, 240).astype(ml_dtypes.float8_e4m3)


def _bf16(a):
    return np.asarray(a, np.float32).astype(ml_dtypes.bfloat16)


def _fold_pt(m):  # [D, D] -> [P, DTl, D] with e = t*128+p on partitions
    return np.ascontiguousarray(m.reshape(DTl, P, D).transpose(1, 0, 2))


def _host_fold(adj_mx, alpha_train, w, d, fc_in_w, fc_in_b):
    """Parameter-only fold-ins (float64 host math), plus the bias field."""
    adj = np.asarray(adj_mx, dtype=np.float64)
    alpha = np.asarray(alpha_train, dtype=np.float64)
    w64 = np.asarray(w, dtype=np.float64)
    d64 = np.asarray(d, dtype=np.float64)
    fcw = np.asarray(fc_in_w, dtype=np.float64)
    fcb = np.asarray(fc_in_b, dtype=np.float64)

    alph = 1.0 / (1.0 + np.exp(-alpha))
    c2 = 0.5 * DT_C * alph  # [N]
    c1 = 1.0 - DT_C - c2  # [N]
    W = DT_C * ((w64 * np.clip(d64, 0.0, 1.0)) @ w64.T)  # [D, D]

    Wp = [np.eye(D)]
    for _ in range(NSTEP):
        Wp.append(Wp[-1] @ W)
    G = [sum(comb(j, k) * Wp[j - k] for j in range(k, NSTEP)) for k in range(NSTEP)]
    R = [fcw.T @ (comb(NSTEP, k) * Wp[NSTEP - k]) + DT_C * G[k] for k in range(NSTEP)]
    R.append(fcw.T.copy())
    bk = [comb(NSTEP, k) * (fcb @ Wp[NSTEP - k]) for k in range(NSTEP)]
    bk.append(fcb.copy())

    gamma = float(np.mean(c1))
    horner = bool(np.ptp(c1) > 1e-9)  # gamma expansion needs constant c1

    extra = {}
    if horner:
        r_pt = np.stack(
            [_fold_pt(R[NSTEP - s]) for s in range(NR)], axis=2
        ).reshape(P, DTl, NR * D)
        rh8 = _q8(SR * r_pt)
        rhat = rh8.astype(np.float64) / SR
        gk = np.array([gamma ** (NSTEP - s) for s in range(NR)])
        rv = r_pt.reshape(P, DTl, NR, D)
        rhv = rhat.reshape(P, DTl, NR, D)
        E = np.einsum("s,ptsd->ptd", gk, rv - rhv)
        S = np.einsum("s,ptsd->ptd", gk, rhv)
        extra["rh8"] = rh8
        extra["e8"] = _q8(SR * E)
        extra["s8"] = _q8(SR / 8.0 * S)
        extra["c1"] = np.ascontiguousarray(c1.reshape(NT, P).T, dtype=np.float32)
        # horner path keeps the baseline's 2^12-scaled adjacency
        adj2t = (ESC_P * c2[:, None] * adj).T
        ajt8 = _q8(np.ascontiguousarray(adj2t.reshape(NT, P, N).transpose(1, 0, 2)))
    else:
        Q = [
            sum(comb(k, j) * gamma ** (k - j) * R[k] for k in range(j, NR))
            for j in range(NJ)
        ]
        # [Q2 | Q1] fp8 moving operand for the products
        qm = np.concatenate([_fold_pt(Q[2]), _fold_pt(Q[1])], axis=-1)
        extra["qm8"] = _q8(SR * qm)
        extra["q0b"] = _bf16(ESC_O * _fold_pt(Q[0]))
        # A2^2 ~= U V via rank-RNK SVD (float32 host math)
        A2 = (c2[:, None] * adj).astype(np.float32)
        B2 = A2 @ A2
        Uf, sf, Vtf = np.linalg.svd(B2)
        U = Uf[:, :RNK]
        V = sf[:RNK, None] * Vtf[:RNK, :]
        # u8[r, nt, n] = U[nt*128+n, r]; v8[m_p, mt, r] = V[r, mt*128+m_p]
        extra["u8"] = _q8(
            SU * np.ascontiguousarray(U.reshape(NT, P, RNK).transpose(2, 0, 1))
        )
        extra["v8"] = _q8(
            SV * np.ascontiguousarray(V.reshape(RNK, NT, P).transpose(2, 1, 0))
        )
        # ESC_O-scaled A2^T for the j=1 term
        adj2t = (ESC_O * c2[:, None] * adj).T
        ajt8 = _q8(np.ascontiguousarray(adj2t.reshape(NT, P, N).transpose(1, 0, 2)))

    # Bias contribution sum_k M^k (1 x b_k), x-independent -> host Horner
    bias_field = None
    if np.any(fcb != 0.0):
        u = np.broadcast_to(bk[NSTEP], (N, D)).copy()
        for k in range(NSTEP - 1, -1, -1):
            u = c1[:, None] * u + c2[:, None] * (adj @ u) + bk[k][None, :]
        bias_field = u.astype(np.float32)

    return horner, ajt8, extra, bias_field


def _in_maps(x, adj_mx, alpha_train, w, d, fc_in_w, fc_in_b):
    horner, ajt8, extra, bias_field = _host_fold(
        adj_mx, alpha_train, w, d, fc_in_w, fc_in_b
    )
    x = np.asarray(x, dtype=np.float64)
    shared = {"ajt8": ajt8, **extra}
    # xt[p, nt, t, b, j] = x[b, n = nt*128+j, e = t*128+p]
    xt_all = x.reshape(NCORES, BL, NT, P, DTl, P).transpose(0, 5, 2, 4, 1, 3)
    xh_all = _q8(SX * xt_all)
    maps = []
    if horner:
        res = SX * xt_all - xh_all.astype(np.float64)
        xl_all = _q8(8.0 * res)
        for c in range(NCORES):
            maps.append(
                {
                    "xh8": np.ascontiguousarray(xh_all[c]),
                    "xl8": np.ascontiguousarray(xl_all[c]),
                    **shared,
                }
            )
    else:
        xbf_all = _bf16(xt_all)
        for c in range(NCORES):
            maps.append(
                {
                    "xh8": np.ascontiguousarray(xh_all[c]),
                    "xbf": np.ascontiguousarray(xbf_all[c]),
                    **shared,
                }
            )
    return maps, bias_field, horner


def run(x, adj_mx, alpha_train, w, d, fc_in_w, fc_in_b, vt=0, **spmd_kwargs):
    maps, bias_field, horner = _in_maps(
        x, adj_mx, alpha_train, w, d, fc_in_w, fc_in_b
    )
    nc = _get_nc(horner)
    res = run_bass_kernel_spmd(
        nc,
        maps,
        core_ids=list(range(NCORES)),
        **spmd_kwargs,
    )
    out = np.concatenate(
        [np.asarray(res.results[c]["out"]).astype(np.float32) for c in range(NCORES)],
        axis=0,
    )
    esc = ESC_P if horner else ESC_O
    out = out * np.float32(1.0 / esc)  # exact power-of-2 descale
    if bias_field is not None:
        out = out + bias_field[None, :, :]
    return out, res


def kernel(x, adj_mx, alpha_train, w, d, fc_in_w, fc_in_b, vt=0):
    out, _ = run(x, adj_mx, alpha_train, w, d, fc_in_w, fc_in_b, vt)
    return out
